# revision 10
# baseline (speedup 1.0000x reference)
"""Multi-head causal self-attention (B=2, S=2048, D=1024, H=16) on 8 TRN2 NeuronCores.

Sharding: data-parallel over batch (2) x tensor-parallel over heads (4 groups of
4 heads). Each core computes Q/K/V projections for its 4 heads, causal
flash-style attention (scores kept transposed [k, q] so no on-chip transposes
are needed), and a partial output projection against its row-slice of W_O.
Host sums the 4 partials per batch and adds the output bias.

Matmuls run in float32r (TF32-like fast path). Softmax denominators come from
an extra all-ones column appended to V (so the P@V matmul also produces the
row sums), and the 1/den broadcast across partitions is done with a K=1 matmul
against a ones vector.
"""

import contextlib
import sys

import numpy as np

sys.path.insert(0, "/opt/trn_rl_repo")

import concourse.bass as bass  # noqa: E402
import concourse.tile as tile  # noqa: E402
from concourse import bacc, mybir  # noqa: E402
from concourse.bass_utils import run_bass_kernel_spmd  # noqa: E402

F32 = mybir.dt.float32
F32R = mybir.dt.float32r
AF = mybir.ActivationFunctionType
ALU = mybir.AluOpType

B, S, D, H = 2, 2048, 1024, 16
DH = D // H          # 64
TPG = 4              # tensor-parallel groups
HPC = H // TPG       # 4 heads per core
CH = HPC * DH        # 256 channels per core
CHA = CH + HPC       # 260: V channels augmented with a ones column per head
NEG = -1.0e9
N_CORES = 8

_PROG = None  # cached compiled Bass program


def _build_program():
    nc = bacc.Bacc("TRN2", target_bir_lowering=False, debug=False,
                   num_devices=N_CORES)

    xT = nc.dram_tensor("xT", [D, S], F32R, kind="ExternalInput").ap()
    wq = nc.dram_tensor("wq", [D, CH], F32R, kind="ExternalInput").ap()
    wk = nc.dram_tensor("wk", [D, CH], F32R, kind="ExternalInput").ap()
    wv = nc.dram_tensor("wv", [D, CHA], F32R, kind="ExternalInput").ap()
    wo = nc.dram_tensor("wo", [CH, D], F32R, kind="ExternalInput").ap()
    bq = nc.dram_tensor("bq", [128, 2], F32, kind="ExternalInput").ap()
    bk = nc.dram_tensor("bk", [128, 2], F32, kind="ExternalInput").ap()
    bv = nc.dram_tensor("bv", [1, CHA], F32R, kind="ExternalInput").ap()
    tri = nc.dram_tensor("tri", [128, 1024], F32, kind="ExternalInput").ap()
    ones = nc.dram_tensor("ones", [1, 128], F32R, kind="ExternalInput").ap()
    out = nc.dram_tensor("out", [S, D], F32, kind="ExternalOutput").ap()

    NQ = S // 512    # 4 q-blocks of 512
    NT = S // 128    # 16 s-tiles / k-blocks

    with tile.TileContext(nc) as tc, contextlib.ExitStack() as ctx:
        const = ctx.enter_context(tc.tile_pool(name="const", bufs=1))
        qt = const.tile([128, 2, S], F32R)     # Q^T/8 (+bq/8): chunk m = heads 2m,2m+1
        kt = const.tile([128, 2, S], F32R)     # K^T (+bk)
        va = const.tile([128, NT, CHA], F32R)  # V augmented: [s, head-major 65-col blocks]
        otn = const.tile([128, 2, S], F32R)    # normalized attention out, transposed
        tri_t = const.tile([128, 1024], F32)
        nc.sync.dma_start(tri_t, tri)
        ones1 = const.tile([1, 128], F32R)
        nc.sync.dma_start(ones1, ones)
        ones64 = ones1[0:1, 0:64]
        bq_t = const.tile([128, 2], F32)
        nc.sync.dma_start(bq_t, bq)
        bk_t = const.tile([128, 2], F32)
        nc.sync.dma_start(bk_t, bk)
        bv_t = const.tile([1, CHA], F32R)
        nc.sync.dma_start(bv_t, bv)

        # ---- phase 1: projections --------------------------------------
        with tc.tile_pool(name="proj", bufs=1) as proj, \
             tc.tile_pool(name="pps", bufs=4, space="PSUM") as pps:
            xt = proj.tile([128, 8, S], F32R)
            wq_t = proj.tile([128, 8, CH], F32R)
            wk_t = proj.tile([128, 8, CH], F32R)
            wv_t = proj.tile([128, 8, CHA], F32R)
            xTr = xT.rearrange("(a p) s -> a p s", p=128)
            wqr = wq.rearrange("(a p) c -> a p c", p=128)
            wkr = wk.rearrange("(a p) c -> a p c", p=128)
            wvr = wv.rearrange("(a p) c -> a p c", p=128)
            for c in range(8):
                nc.sync.dma_start(wq_t[:, c, :], wqr[c])
                nc.sync.dma_start(wk_t[:, c, :], wkr[c])
                nc.sync.dma_start(wv_t[:, c, :], wvr[c])
                nc.sync.dma_start(xt[:, c, :], xTr[c])

            # Q^T and K^T: stationary = weight chunk, moving = x^T
            for w_t, dst, bias_t, scaled in ((wq_t, qt, bq_t, True),
                                             (wk_t, kt, bk_t, False)):
                for m in range(2):
                    for n in range(NQ):
                        ps = pps.tile([128, 512], F32, tag="ps")
                        for c in range(8):
                            nc.tensor.matmul(
                                ps, (w_t[:, c, m * 128:(m + 1) * 128]),
                                (xt[:, c, n * 512:(n + 1) * 512]),
                                start=(c == 0), stop=(c == 7))
                        dstv = dst[:, m, n * 512:(n + 1) * 512]
                        if scaled:  # fold the 1/sqrt(DH) into Q
                            nc.vector.tensor_scalar(
                                dstv, ps, 0.125, bias_t[:, m:m + 1],
                                op0=ALU.mult, op1=ALU.add)
                        else:
                            nc.vector.tensor_scalar(
                                dstv, ps, bias_t[:, m:m + 1], None, op0=ALU.add)

            # V (not transposed): stationary = x^T tile, moving = wv_aug
            for t in range(NT):
                ps = pps.tile([128, 512], F32, tag="ps")
                psv = ps[:, 0:CHA]
                for c in range(8):
                    nc.tensor.matmul(
                        psv, (xt[:, c, t * 128:(t + 1) * 128]),
                        (wv_t[:, c, :]), start=(c == 0), stop=False)
                # bias row (contains the 1.0 for the ones columns)
                nc.tensor.matmul(psv, (ones1), (bv_t), start=False,
                                 stop=True)
                nc.vector.tensor_copy(va[:, t, :], psv)

        # ---- phase 2: attention ----------------------------------------
        with tc.tile_pool(name="sm", bufs=4) as sm, \
             tc.tile_pool(name="stp", bufs=2, space="PSUM") as stp, \
             tc.tile_pool(name="pvp", bufs=2, space="PSUM") as pvp, \
             tc.tile_pool(name="bcp", bufs=1, space="PSUM") as bcp:
            for p in range(2):          # head pair = channel chunk
                for j in range(NQ):     # q-block of 512
                    nkb = 4 * (j + 1)   # causal: k-blocks 0..nkb-1
                    qsl = slice(j * 512, (j + 1) * 512)
                    pv = [pvp.tile([65, 512], F32, tag="pv", name=f"pv{_hh}")
                          for _hh in range(2)]
                    for g in range(nkb // 2):
                        st = [stp.tile([128, 1024], F32, tag="st", name=f"st{_hh}")
                              for _hh in range(2)]
                        for i in range(2):
                            kb = 2 * g + i
                            for hh in range(2):  # packed: row groups 0-63/64-127
                                oh = hh * 64
                                nc.tensor.matmul(
                                    st[hh][:, i * 512:(i + 1) * 512],
                                    (kt[oh:oh + 64, p, kb * 128:(kb + 1) * 128]),
                                    (qt[oh:oh + 64, p, qsl]),
                                    start=True, stop=True)
                        for i in range(2):
                            kb = 2 * g + i
                            rel = kb * 128 - j * 512
                            if rel >= 0:
                                # causal bias over cols [0, rel+128): fully
                                # masked left part + triangular strip, via a
                                # slice of the staircase tile
                                # tri[kk, x] = -1e9 if x < 512+kk else 0
                                for hh in range(2):
                                    sl = st[hh][:, i * 512:i * 512 + rel + 128]
                                    nc.vector.tensor_add(
                                        sl, sl, tri_t[:, 512 - rel:640])
                        pt = [None, None]
                        for hh in range(2):
                            pt[hh] = sm.tile([128, 1024], F32R, tag="pt", name=f"pt{hh}")
                            nc.scalar.activation(pt[hh], st[hh], AF.Exp)
                        for i in range(2):
                            kb = 2 * g + i
                            for hh in range(2):
                                h = 2 * p + hh
                                nc.tensor.matmul(
                                    pv[hh], (va[:, kb, h * 65:h * 65 + 65]),
                                    (pt[hh][:, i * 512:(i + 1) * 512]),
                                    start=(kb == 0), stop=(kb == nkb - 1),
                                    skip_group_check=True)
                    for hh in range(2):
                        oh = hh * 64
                        rec = sm.tile([1, 512], F32R, tag="rec")
                        with nc.allow_low_precision(reason="f32r = fp32 bits"):
                            nc.vector.reciprocal(rec, pv[hh][64:65, :])
                        bc = bcp.tile([64, 512], F32, tag="bc")
                        nc.tensor.matmul(bc, (ones64), (rec), start=True,
                                         stop=True)
                        bcs = sm.tile([64, 512], F32, tag="bcs")
                        nc.scalar.activation(bcs, bc, AF.Copy)
                        nc.vector.tensor_mul(otn[oh:oh + 64, p, qsl],
                                             pv[hh][0:64, :], bcs)

        # ---- phase 3: output projection (partial; host reduces) --------
        with tc.tile_pool(name="wop", bufs=1) as wop, \
             tc.tile_pool(name="ost", bufs=4) as ost, \
             tc.tile_pool(name="ops", bufs=4, space="PSUM") as ops:
            wo_t = wop.tile([128, 2, D], F32R)
            wor = wo.rearrange("(a p) n -> a p n", p=128)
            for c2 in range(2):
                nc.sync.dma_start(wo_t[:, c2, :], wor[c2])
            for t in range(NT):
                for n in range(2):
                    ps = ops.tile([128, 512], F32, tag="ops")
                    for c2 in range(2):
                        nc.tensor.matmul(
                            ps, (otn[:, c2, t * 128:(t + 1) * 128]),
                            (wo_t[:, c2, n * 512:(n + 1) * 512]),
                            start=(c2 == 0), stop=(c2 == 1))
                    so = ost.tile([128, 512], F32, tag="so")
                    if n == 0:
                        nc.scalar.activation(so, ps, AF.Copy)
                    else:
                        nc.vector.tensor_copy(so, ps)
                    nc.sync.dma_start(out[t * 128:(t + 1) * 128,
                                          n * 512:(n + 1) * 512], so)

    nc.compile()
    return nc


def _tri_np():
    # staircase causal bias: tri[kk, x] = NEG if x < 512+kk else 0
    xs = np.arange(1024)[None, :]
    ks = np.arange(128)[:, None]
    return np.where(xs < 512 + ks, np.float32(NEG),
                    np.float32(0.0)).astype(np.float32)


def build_in_maps(x, Wq, bq, Wk, bk, Wv, bv, Wo):
    tri_np = _tri_np()
    ones_np = np.ones((1, 128), dtype=np.float32)
    xT_b = [np.ascontiguousarray(x[b].T) for b in range(B)]
    in_maps = []
    for c in range(N_CORES):
        b, tp = divmod(c, TPG)
        sl = slice(tp * CH, (tp + 1) * CH)
        wv_aug = np.zeros((D, CHA), dtype=np.float32)
        bv_aug = np.zeros((1, CHA), dtype=np.float32)
        for h in range(HPC):
            hsl = slice(tp * CH + h * DH, tp * CH + (h + 1) * DH)
            wv_aug[:, h * 65:h * 65 + DH] = Wv[:, hsl]
            bv_aug[0, h * 65:h * 65 + DH] = bv[hsl]
            bv_aug[0, h * 65 + DH] = 1.0
        in_maps.append({
            "xT": xT_b[b],
            "wq": np.ascontiguousarray(Wq[:, sl], dtype=np.float32),
            "wk": np.ascontiguousarray(Wk[:, sl], dtype=np.float32),
            "wv": wv_aug,
            "wo": np.ascontiguousarray(Wo[sl, :], dtype=np.float32),
            "bq": (bq[sl].astype(np.float32) * 0.125).reshape(2, 128).T.copy(),
            "bk": bk[sl].astype(np.float32).reshape(2, 128).T.copy(),
            "bv": bv_aug,
            "tri": tri_np,
            "ones": ones_np,
        })
    return in_maps


def _get_program():
    global _PROG
    if _PROG is None:
        _PROG = _build_program()
    return _PROG


def kernel(x, mask, Wq, bq, Wk, bk, Wv, bv, Wo, bo):
    x = np.asarray(x, dtype=np.float32)
    mask = np.asarray(mask)
    causal = bool(
        np.array_equal(mask != 0,
                       np.tril(np.ones((S, S), dtype=bool))))
    if not causal:
        # Fallback for non-causal masks: exact host computation.
        q = (x @ Wq + bq).reshape(B, S, H, DH).transpose(0, 2, 1, 3)
        k = (x @ Wk + bk).reshape(B, S, H, DH).transpose(0, 2, 1, 3)
        v = (x @ Wv + bv).reshape(B, S, H, DH).transpose(0, 2, 1, 3)
        attn = np.einsum("bhqd,bhkd->bhqk", q, k) / np.sqrt(np.float32(DH))
        attn = np.where(mask == 0, np.float32(-1e9), attn)
        attn = attn - attn.max(axis=-1, keepdims=True)
        e = np.exp(attn)
        p = e / e.sum(axis=-1, keepdims=True)
        o = np.einsum("bhqk,bhkd->bhqd", p, v)
        o = o.transpose(0, 2, 1, 3).reshape(B, S, D)
        return (o @ Wo + bo).astype(np.float32)

    nc = _get_program()
    in_maps = build_in_maps(x, Wq, bq, Wk, bk, Wv, bv, Wo)
    res = run_bass_kernel_spmd(nc, in_maps, core_ids=list(range(N_CORES)))
    out = np.zeros((B, S, D), dtype=np.float32)
    for c in range(N_CORES):
        out[c // TPG] += res.results[c]["out"]
    out += bo.astype(np.float32)
    return out


# revision 18
# speedup vs baseline: 1.1384x; 1.1384x over previous
"""Multi-head causal self-attention (B=2, S=2048, D=1024, H=16) on 8 TRN2 NeuronCores.

Sharding: data-parallel over batch (2) x tensor-parallel over heads (4 groups of
4 heads). Each core computes Q/K/V projections for its 4 heads, causal
flash-style attention (scores kept transposed [k, q] so no on-chip transposes
are needed), and a partial output projection against its row-slice of W_O.
Host sums the 4 partials per batch and adds the output bias.

Matmuls run in float32r (TF32-like fast path, ~2.5e-4 end-to-end rel err).
Softmax denominators come from an extra all-ones column appended to V (so the
P@V matmul also produces the row sums); the per-query 1/den is broadcast
across partitions via a DRAM-bounce DMA with a partition-step-0 source AP.
Measured: ~314 us per core (max over 8 cores), all 8 cores within ~2%.
"""

import contextlib
import sys

import numpy as np

sys.path.insert(0, "/opt/trn_rl_repo")

import concourse.bass as bass  # noqa: E402
import concourse.tile as tile  # noqa: E402
from concourse import bacc, mybir  # noqa: E402
from concourse.bass_utils import run_bass_kernel_spmd  # noqa: E402

F32 = mybir.dt.float32
F32R = mybir.dt.float32r
AF = mybir.ActivationFunctionType
ALU = mybir.AluOpType

B, S, D, H = 2, 2048, 1024, 16
DH = D // H          # 64
TPG = 4              # tensor-parallel groups
HPC = H // TPG       # 4 heads per core
CH = HPC * DH        # 256 channels per core
CHA = CH + HPC       # 260: V channels augmented with a ones column per head
NEG = -1.0e9
N_CORES = 8

_PROG = None  # cached compiled Bass program


def _build_program():
    nc = bacc.Bacc("TRN2", target_bir_lowering=False, debug=False,
                   num_devices=N_CORES)

    xT = nc.dram_tensor("xT", [D, S], F32R, kind="ExternalInput").ap()
    wq = nc.dram_tensor("wq", [D, CH], F32R, kind="ExternalInput").ap()
    wk = nc.dram_tensor("wk", [D, CH], F32R, kind="ExternalInput").ap()
    wv = nc.dram_tensor("wv", [D, CHA], F32R, kind="ExternalInput").ap()
    wo = nc.dram_tensor("wo", [CH, D], F32R, kind="ExternalInput").ap()
    bq = nc.dram_tensor("bq", [128, 2], F32, kind="ExternalInput").ap()
    bk = nc.dram_tensor("bk", [128, 2], F32, kind="ExternalInput").ap()
    bv = nc.dram_tensor("bv", [1, CHA], F32R, kind="ExternalInput").ap()
    tri = nc.dram_tensor("tri", [128, 1024], F32, kind="ExternalInput").ap()
    ones = nc.dram_tensor("ones", [1, 128], F32R, kind="ExternalInput").ap()
    out = nc.dram_tensor("out", [S, D], F32, kind="ExternalOutput").ap()

    NQ = S // 512    # 4 q-blocks of 512
    NT = S // 128    # 16 s-tiles / k-blocks

    with tile.TileContext(nc) as tc, contextlib.ExitStack() as ctx:
        const = ctx.enter_context(tc.tile_pool(name="const", bufs=1))
        qt = const.tile([128, 2, S], F32R)     # Q^T/8 (+bq/8): chunk m = heads 2m,2m+1
        kt = const.tile([128, 2, S], F32R)     # K^T (+bk)
        va = const.tile([128, NT, CHA], F32R)  # V augmented: [s, head-major 65-col blocks]
        otn = const.tile([128, 2, S], F32R)    # normalized attention out, transposed
        tri_t = const.tile([128, 1024], F32)
        nc.sync.dma_start(tri_t, tri)
        ones1 = const.tile([1, 128], F32R)
        nc.sync.dma_start(ones1, ones)
        bq_t = const.tile([128, 2], F32)
        nc.sync.dma_start(bq_t, bq)
        bk_t = const.tile([128, 2], F32)
        nc.sync.dma_start(bk_t, bk)
        bv_t = const.tile([1, CHA], F32R)
        nc.sync.dma_start(bv_t, bv)
        wo_t = const.tile([128, 2, D], F32R)
        wor = wo.rearrange("(a p) n -> a p n", p=128)
        for c2 in range(2):
            nc.sync.dma_start(wo_t[:, c2, :], wor[c2])

        # ---- phase 1: projections --------------------------------------
        with tc.tile_pool(name="proj", bufs=1) as proj, \
             tc.tile_pool(name="pps", bufs=4, space="PSUM") as pps:
            xt = proj.tile([128, 8, S], F32R)
            wq_t = proj.tile([128, 8, CH], F32R)
            wk_t = proj.tile([128, 8, CH], F32R)
            wv_t = proj.tile([128, 8, CHA], F32R)
            xTr = xT.rearrange("(a p) s -> a p s", p=128)
            wqr = wq.rearrange("(a p) c -> a p c", p=128)
            wkr = wk.rearrange("(a p) c -> a p c", p=128)
            wvr = wv.rearrange("(a p) c -> a p c", p=128)
            for c in range(8):
                nc.sync.dma_start(wq_t[:, c, :], wqr[c])
                nc.sync.dma_start(wk_t[:, c, :], wkr[c])
                nc.sync.dma_start(wv_t[:, c, :], wvr[c])
                nc.sync.dma_start(xt[:, c, :], xTr[c])

            def qk_proj(w_t, dst, bias_t, scaled, m):
                # stationary = weight chunk, moving = x^T; c-outer so the
                # in-order PE starts as soon as each DMA chunk lands
                pss = [pps.tile([128, 512], F32, tag="ps", name=f"ps{_n}")
                       for _n in range(NQ)]
                for c in range(8):
                    for n in range(NQ):
                        nc.tensor.matmul(
                            pss[n], (w_t[:, c, m * 128:(m + 1) * 128]),
                            (xt[:, c, n * 512:(n + 1) * 512]),
                            start=(c == 0), stop=(c == 7),
                            skip_group_check=True)
                for n in range(NQ):
                    dstv = dst[:, m, n * 512:(n + 1) * 512]
                    if scaled:  # fold the 1/sqrt(DH) into Q
                        nc.vector.tensor_scalar(
                            dstv, pss[n], 0.125, bias_t[:, m:m + 1],
                            op0=ALU.mult, op1=ALU.add)
                    else:
                        nc.vector.tensor_scalar(
                            dstv, pss[n], bias_t[:, m:m + 1], None,
                            op0=ALU.add)

            def v_proj():
                # V (not transposed): stationary = x^T tile, moving = wv_aug
                for tb in range(NT // 4):
                    pss = [pps.tile([128, 512], F32, tag="ps", name=f"ps{_n}")
                           for _n in range(4)]
                    for c in range(8):
                        for i in range(4):
                            t = tb * 4 + i
                            nc.tensor.matmul(
                                pss[i][:, 0:CHA],
                                (xt[:, c, t * 128:(t + 1) * 128]),
                                (wv_t[:, c, :]), start=(c == 0), stop=False,
                                skip_group_check=True)
                    for i in range(4):
                        t = tb * 4 + i
                        # bias row (contains the 1.0 for the ones columns)
                        nc.tensor.matmul(pss[i][:, 0:CHA], (ones1), (bv_t),
                                         start=False, stop=True,
                                         skip_group_check=True)
                        nc.vector.tensor_copy(va[:, t, :], pss[i][:, 0:CHA])

            # pair-0 inputs first so attention can overlap chunk-1 projections
            qk_proj(wq_t, qt, bq_t, True, 0)
            qk_proj(wk_t, kt, bk_t, False, 0)
            v_proj()
            qk_proj(wq_t, qt, bq_t, True, 1)
            qk_proj(wk_t, kt, bk_t, False, 1)

        # ---- phase 2+3: attention with interleaved output projection ---
        with tc.tile_pool(name="sm", bufs=4) as sm, \
             tc.tile_pool(name="ost", bufs=4) as ost, \
             tc.tile_pool(name="stp", bufs=2, space="PSUM") as stp, \
             tc.tile_pool(name="pvp", bufs=2, space="PSUM") as pvp, \
             tc.tile_pool(name="ops", bufs=2, space="PSUM") as ops, \
             tc.tile_pool(name="dsp", bufs=4, space="DRAM") as dsp:
            for j in range(NQ):         # q-block of 512
                nkb = 4 * (j + 1)       # causal: k-blocks 0..nkb-1
                qsl = slice(j * 512, (j + 1) * 512)
                for p in range(2):      # head pair = channel chunk
                    pv = [pvp.tile([65, 512], F32, tag="pv", name=f"pv{_hh}")
                          for _hh in range(2)]
                    for g in range(nkb // 2):
                        st = [stp.tile([128, 1024], F32, tag="st",
                                       name=f"st{_hh}") for _hh in range(2)]
                        for i in range(2):
                            kb = 2 * g + i
                            for hh in range(2):  # packed rows 0-63/64-127
                                oh = hh * 64
                                nc.tensor.matmul(
                                    st[hh][:, i * 512:(i + 1) * 512],
                                    (kt[oh:oh + 64, p, kb * 128:(kb + 1) * 128]),
                                    (qt[oh:oh + 64, p, qsl]),
                                    start=True, stop=True)
                        for i in range(2):
                            kb = 2 * g + i
                            rel = kb * 128 - j * 512
                            if rel >= 0:
                                # causal staircase bias over cols [0, rel+128)
                                for hh in range(2):
                                    sl = st[hh][:, i * 512:i * 512 + rel + 128]
                                    nc.vector.tensor_add(
                                        sl, sl, tri_t[:, 512 - rel:640])
                        pt = [None, None]
                        for hh in range(2):
                            pt[hh] = sm.tile([128, 1024], F32R, tag="pt",
                                             name=f"pt{hh}")
                            nc.scalar.activation(pt[hh], st[hh], AF.Exp)
                        for i in range(2):
                            kb = 2 * g + i
                            for hh in range(2):
                                h = 2 * p + hh
                                nc.tensor.matmul(
                                    pv[hh], (va[:, kb, h * 65:h * 65 + 65]),
                                    (pt[hh][:, i * 512:(i + 1) * 512]),
                                    start=(kb == 0), stop=(kb == nkb - 1),
                                    skip_group_check=True)
                    for hh in range(2):
                        oh = hh * 64
                        rec = sm.tile([1, 512], F32, tag="rec")
                        nc.vector.reciprocal(rec, pv[hh][64:65, :])
                        # broadcast 1/den across 64 partitions via DRAM bounce
                        drow = dsp.tile([1, 512], F32, tag="ds", name="ds")
                        nc.sync.dma_start(drow, rec)
                        bcast_src = bass.AP(
                            tensor=drow.tensor, offset=drow.offset,
                            ap=[[0, 64]] + list(drow.ap)[1:])
                        bcs = sm.tile([64, 512], F32, tag="bcs")
                        nc.sync.dma_start(bcs, bcast_src)
                        nc.vector.tensor_mul(otn[oh:oh + 64, p, qsl],
                                             pv[hh][0:64, :], bcs)
                # output projection for this q-block (partial; host reduces)
                for t in range(4 * j, 4 * (j + 1)):
                    for n in range(2):
                        ps = ops.tile([128, 512], F32, tag="ops", name="ops")
                        for c2 in range(2):
                            nc.tensor.matmul(
                                ps, (otn[:, c2, t * 128:(t + 1) * 128]),
                                (wo_t[:, c2, n * 512:(n + 1) * 512]),
                                start=(c2 == 0), stop=(c2 == 1))
                        so = ost.tile([128, 512], F32, tag="so", name="so")
                        if n == 0:
                            nc.scalar.activation(so, ps, AF.Copy)
                        else:
                            nc.vector.tensor_copy(so, ps)
                        nc.sync.dma_start(out[t * 128:(t + 1) * 128,
                                              n * 512:(n + 1) * 512], so)

    nc.compile()
    return nc


def _tri_np():
    # staircase causal bias: tri[kk, x] = NEG if x < 512+kk else 0
    xs = np.arange(1024)[None, :]
    ks = np.arange(128)[:, None]
    return np.where(xs < 512 + ks, np.float32(NEG),
                    np.float32(0.0)).astype(np.float32)


def build_in_maps(x, Wq, bq, Wk, bk, Wv, bv, Wo):
    tri_np = _tri_np()
    ones_np = np.ones((1, 128), dtype=np.float32)
    xT_b = [np.ascontiguousarray(x[b].T) for b in range(B)]
    in_maps = []
    for c in range(N_CORES):
        b, tp = divmod(c, TPG)
        sl = slice(tp * CH, (tp + 1) * CH)
        wv_aug = np.zeros((D, CHA), dtype=np.float32)
        bv_aug = np.zeros((1, CHA), dtype=np.float32)
        for h in range(HPC):
            hsl = slice(tp * CH + h * DH, tp * CH + (h + 1) * DH)
            wv_aug[:, h * 65:h * 65 + DH] = Wv[:, hsl]
            bv_aug[0, h * 65:h * 65 + DH] = bv[hsl]
            bv_aug[0, h * 65 + DH] = 1.0
        in_maps.append({
            "xT": xT_b[b],
            "wq": np.ascontiguousarray(Wq[:, sl], dtype=np.float32),
            "wk": np.ascontiguousarray(Wk[:, sl], dtype=np.float32),
            "wv": wv_aug,
            "wo": np.ascontiguousarray(Wo[sl, :], dtype=np.float32),
            "bq": (bq[sl].astype(np.float32) * 0.125).reshape(2, 128).T.copy(),
            "bk": bk[sl].astype(np.float32).reshape(2, 128).T.copy(),
            "bv": bv_aug,
            "tri": tri_np,
            "ones": ones_np,
        })
    return in_maps


def _get_program():
    global _PROG
    if _PROG is None:
        _PROG = _build_program()
    return _PROG


def kernel(x, mask, Wq, bq, Wk, bk, Wv, bv, Wo, bo):
    x = np.asarray(x, dtype=np.float32)
    mask = np.asarray(mask)
    Wq, Wk, Wv, Wo = (np.asarray(w, dtype=np.float32)
                      for w in (Wq, Wk, Wv, Wo))
    bq, bk, bv, bo = (np.asarray(b, dtype=np.float32)
                      for b in (bq, bk, bv, bo))
    causal = bool(
        np.array_equal(mask != 0,
                       np.tril(np.ones((S, S), dtype=bool))))
    if not causal:
        # Fallback for non-causal masks: exact host computation.
        q = (x @ Wq + bq).reshape(B, S, H, DH).transpose(0, 2, 1, 3)
        k = (x @ Wk + bk).reshape(B, S, H, DH).transpose(0, 2, 1, 3)
        v = (x @ Wv + bv).reshape(B, S, H, DH).transpose(0, 2, 1, 3)
        attn = np.einsum("bhqd,bhkd->bhqk", q, k) / np.sqrt(np.float32(DH))
        attn = np.where(mask == 0, np.float32(-1e9), attn)
        attn = attn - attn.max(axis=-1, keepdims=True)
        e = np.exp(attn)
        p = e / e.sum(axis=-1, keepdims=True)
        o = np.einsum("bhqk,bhkd->bhqd", p, v)
        o = o.transpose(0, 2, 1, 3).reshape(B, S, D)
        return (o @ Wo + bo).astype(np.float32)

    nc = _get_program()
    in_maps = build_in_maps(x, Wq, bq, Wk, bk, Wv, bv, Wo)
    res = run_bass_kernel_spmd(nc, in_maps, core_ids=list(range(N_CORES)))
    out = np.zeros((B, S, D), dtype=np.float32)
    for c in range(N_CORES):
        out[c // TPG] += res.results[c]["out"]
    out += bo.astype(np.float32)
    return out


# revision 19
# speedup vs baseline: 1.1893x; 1.0447x over previous
"""Multi-head causal self-attention (B=2, S=2048, D=1024, H=16) on 8 TRN2 NeuronCores.

Sharding: data-parallel over batch (2) x tensor-parallel over heads (4 groups of
4 heads). Each core computes Q/K/V projections for its 4 heads, causal
flash-style attention (scores kept transposed [k, q] so no on-chip transposes
are needed), and a partial output projection against its row-slice of W_O.
Host sums the 4 partials per batch and adds the output bias.

Matmuls run in float32r (TF32-like fast path, ~2.5e-4 end-to-end rel err).
Softmax denominators come from an extra all-ones column appended to V (so the
P@V matmul also produces the row sums); the per-query 1/den is broadcast
across partitions via a DRAM-bounce DMA with a partition-step-0 source AP.
Measured: ~312 us max-core / ~305 us mean over 8 cores (all within ~2%).
"""

import contextlib
import sys

import numpy as np

sys.path.insert(0, "/opt/trn_rl_repo")

import concourse.bass as bass  # noqa: E402
import concourse.tile as tile  # noqa: E402
from concourse import bacc, mybir  # noqa: E402
from concourse.bass_utils import run_bass_kernel_spmd  # noqa: E402

F32 = mybir.dt.float32
F32R = mybir.dt.float32r
AF = mybir.ActivationFunctionType
ALU = mybir.AluOpType

B, S, D, H = 2, 2048, 1024, 16
DH = D // H          # 64
TPG = 4              # tensor-parallel groups
HPC = H // TPG       # 4 heads per core
CH = HPC * DH        # 256 channels per core
CHA = CH + HPC       # 260: V channels augmented with a ones column per head
NEG = -1.0e9
N_CORES = 8

_PROG = None  # cached compiled Bass program


def _build_program():
    nc = bacc.Bacc("TRN2", target_bir_lowering=False, debug=False,
                   num_devices=N_CORES)

    xT = nc.dram_tensor("xT", [D, S], F32R, kind="ExternalInput").ap()
    wq = nc.dram_tensor("wq", [D, CH], F32R, kind="ExternalInput").ap()
    wk = nc.dram_tensor("wk", [D, CH], F32R, kind="ExternalInput").ap()
    wv = nc.dram_tensor("wv", [D, CHA], F32R, kind="ExternalInput").ap()
    wo = nc.dram_tensor("wo", [CH, D], F32R, kind="ExternalInput").ap()
    bq = nc.dram_tensor("bq", [128, 2], F32, kind="ExternalInput").ap()
    bk = nc.dram_tensor("bk", [128, 2], F32, kind="ExternalInput").ap()
    bv = nc.dram_tensor("bv", [1, CHA], F32R, kind="ExternalInput").ap()
    tri = nc.dram_tensor("tri", [128, 1024], F32, kind="ExternalInput").ap()
    ones = nc.dram_tensor("ones", [1, 128], F32R, kind="ExternalInput").ap()
    out = nc.dram_tensor("out", [S, D], F32, kind="ExternalOutput").ap()

    NQ = S // 512    # 4 q-blocks of 512
    NT = S // 128    # 16 s-tiles / k-blocks

    with tile.TileContext(nc) as tc, contextlib.ExitStack() as ctx:
        const = ctx.enter_context(tc.tile_pool(name="const", bufs=1))
        qt = const.tile([128, 2, S], F32R)     # Q^T/8 (+bq/8): chunk m = heads 2m,2m+1
        kt = const.tile([128, 2, S], F32R)     # K^T (+bk)
        va = const.tile([128, NT, CHA], F32R)  # V augmented: [s, head-major 65-col blocks]
        otn = const.tile([128, 2, S], F32R)    # normalized attention out, transposed
        tri_t = const.tile([128, 1024], F32)
        nc.sync.dma_start(tri_t, tri)
        ones1 = const.tile([1, 128], F32R)
        nc.sync.dma_start(ones1, ones)
        bq_t = const.tile([128, 2], F32)
        nc.sync.dma_start(bq_t, bq)
        bk_t = const.tile([128, 2], F32)
        nc.sync.dma_start(bk_t, bk)
        bv_t = const.tile([1, CHA], F32R)
        nc.sync.dma_start(bv_t, bv)
        wo_t = const.tile([128, 2, D], F32R)
        wor = wo.rearrange("(a p) n -> a p n", p=128)
        for c2 in range(2):
            nc.sync.dma_start(wo_t[:, c2, :], wor[c2])

        # ---- phase 1: projections --------------------------------------
        with tc.tile_pool(name="proj", bufs=1) as proj, \
             tc.tile_pool(name="pps", bufs=4, space="PSUM") as pps:
            xt = proj.tile([128, 8, S], F32R)
            wq_t = proj.tile([128, 8, CH], F32R)
            wk_t = proj.tile([128, 8, CH], F32R)
            wv_t = proj.tile([128, 8, CHA], F32R)
            xTr = xT.rearrange("(a p) s -> a p s", p=128)
            wqr = wq.rearrange("(a p) c -> a p c", p=128)
            wkr = wk.rearrange("(a p) c -> a p c", p=128)
            wvr = wv.rearrange("(a p) c -> a p c", p=128)
            for c in range(8):
                nc.sync.dma_start(wq_t[:, c, :], wqr[c])
                nc.sync.dma_start(wk_t[:, c, :], wkr[c])
                nc.sync.dma_start(wv_t[:, c, :], wvr[c])
                nc.sync.dma_start(xt[:, c, :], xTr[c])

            def qk_proj(w_t, dst, bias_t, scaled, m):
                # stationary = weight chunk, moving = x^T; c-outer so the
                # in-order PE starts as soon as each DMA chunk lands
                pss = [pps.tile([128, 512], F32, tag="ps", name=f"ps{_n}")
                       for _n in range(NQ)]
                for c in range(8):
                    for n in range(NQ):
                        nc.tensor.matmul(
                            pss[n], (w_t[:, c, m * 128:(m + 1) * 128]),
                            (xt[:, c, n * 512:(n + 1) * 512]),
                            start=(c == 0), stop=(c == 7),
                            skip_group_check=True)
                for n in range(NQ):
                    dstv = dst[:, m, n * 512:(n + 1) * 512]
                    if scaled:  # fold the 1/sqrt(DH) into Q
                        nc.vector.tensor_scalar(
                            dstv, pss[n], 0.125, bias_t[:, m:m + 1],
                            op0=ALU.mult, op1=ALU.add)
                    else:
                        nc.vector.tensor_scalar(
                            dstv, pss[n], bias_t[:, m:m + 1], None,
                            op0=ALU.add)

            def v_proj():
                # V (not transposed): stationary = x^T tile, moving = wv_aug
                for tb in range(NT // 4):
                    pss = [pps.tile([128, 512], F32, tag="ps", name=f"ps{_n}")
                           for _n in range(4)]
                    for c in range(8):
                        for i in range(4):
                            t = tb * 4 + i
                            nc.tensor.matmul(
                                pss[i][:, 0:CHA],
                                (xt[:, c, t * 128:(t + 1) * 128]),
                                (wv_t[:, c, :]), start=(c == 0), stop=False,
                                skip_group_check=True)
                    for i in range(4):
                        t = tb * 4 + i
                        # bias row (contains the 1.0 for the ones columns)
                        nc.tensor.matmul(pss[i][:, 0:CHA], (ones1), (bv_t),
                                         start=False, stop=True,
                                         skip_group_check=True)
                        nc.vector.tensor_copy(va[:, t, :], pss[i][:, 0:CHA])

            # pair-0 inputs first so attention can overlap chunk-1 projections
            qk_proj(wq_t, qt, bq_t, True, 0)
            qk_proj(wk_t, kt, bk_t, False, 0)
            v_proj()
            qk_proj(wq_t, qt, bq_t, True, 1)
            qk_proj(wk_t, kt, bk_t, False, 1)

        # ---- phase 2+3: attention with interleaved output projection ---
        with tc.tile_pool(name="sm", bufs=4) as sm, \
             tc.tile_pool(name="ost", bufs=4) as ost, \
             tc.tile_pool(name="stp", bufs=2, space="PSUM") as stp, \
             tc.tile_pool(name="pvp", bufs=2, space="PSUM") as pvp, \
             tc.tile_pool(name="ops", bufs=2, space="PSUM") as ops, \
             tc.tile_pool(name="dsp", bufs=4, space="DRAM") as dsp:
            for j in range(NQ):         # q-block of 512
                nkb = 4 * (j + 1)       # causal: k-blocks 0..nkb-1
                qsl = slice(j * 512, (j + 1) * 512)
                for p in range(2):      # head pair = channel chunk
                    pv = [pvp.tile([65, 512], F32, tag="pv", name=f"pv{_hh}")
                          for _hh in range(2)]
                    for g in range(nkb // 2):
                        st = [stp.tile([128, 1024], F32, tag="st",
                                       name=f"st{_hh}") for _hh in range(2)]
                        for i in range(2):
                            kb = 2 * g + i
                            for hh in range(2):  # packed rows 0-63/64-127
                                oh = hh * 64
                                nc.tensor.matmul(
                                    st[hh][:, i * 512:(i + 1) * 512],
                                    (kt[oh:oh + 64, p, kb * 128:(kb + 1) * 128]),
                                    (qt[oh:oh + 64, p, qsl]),
                                    start=True, stop=True)
                        for i in range(2):
                            kb = 2 * g + i
                            rel = kb * 128 - j * 512
                            if rel >= 0:
                                # causal staircase bias over cols [0, rel+128)
                                for hh in range(2):
                                    sl = st[hh][:, i * 512:i * 512 + rel + 128]
                                    nc.vector.tensor_add(
                                        sl, sl, tri_t[:, 512 - rel:640])
                        pt = [None, None]
                        for hh in range(2):
                            pt[hh] = sm.tile([128, 1024], F32R, tag="pt",
                                             name=f"pt{hh}")
                            nc.scalar.activation(pt[hh], st[hh], AF.Exp)
                        for i in range(2):
                            kb = 2 * g + i
                            for hh in range(2):
                                h = 2 * p + hh
                                nc.tensor.matmul(
                                    pv[hh], (va[:, kb, h * 65:h * 65 + 65]),
                                    (pt[hh][:, i * 512:(i + 1) * 512]),
                                    start=(kb == 0), stop=(kb == nkb - 1),
                                    skip_group_check=True)
                    for hh in range(2):
                        oh = hh * 64
                        rec = sm.tile([1, 512], F32, tag="rec")
                        nc.vector.reciprocal(rec, pv[hh][64:65, :])
                        # broadcast 1/den across 64 partitions via DRAM bounce
                        drow = dsp.tile([1, 512], F32, tag="ds", name="ds")
                        nc.sync.dma_start(drow, rec)
                        bcast_src = bass.AP(
                            tensor=drow.tensor, offset=drow.offset,
                            ap=[[0, 64]] + list(drow.ap)[1:])
                        bcs = sm.tile([64, 512], F32, tag="bcs")
                        nc.sync.dma_start(bcs, bcast_src)
                        nc.vector.tensor_mul(otn[oh:oh + 64, p, qsl],
                                             pv[hh][0:64, :], bcs)
                # output projection for this q-block (partial; host reduces)
                for t in range(4 * j, 4 * (j + 1)):
                    for n in range(2):
                        ps = ops.tile([128, 512], F32, tag="ops", name="ops")
                        for c2 in range(2):
                            nc.tensor.matmul(
                                ps, (otn[:, c2, t * 128:(t + 1) * 128]),
                                (wo_t[:, c2, n * 512:(n + 1) * 512]),
                                start=(c2 == 0), stop=(c2 == 1))
                        so = ost.tile([128, 512], F32, tag="so", name="so")
                        if n == 0:
                            nc.scalar.activation(so, ps, AF.Copy)
                        else:
                            nc.vector.tensor_copy(so, ps)
                        nc.sync.dma_start(out[t * 128:(t + 1) * 128,
                                              n * 512:(n + 1) * 512], so)

    nc.compile()
    return nc


def _tri_np():
    # staircase causal bias: tri[kk, x] = NEG if x < 512+kk else 0
    xs = np.arange(1024)[None, :]
    ks = np.arange(128)[:, None]
    return np.where(xs < 512 + ks, np.float32(NEG),
                    np.float32(0.0)).astype(np.float32)


def build_in_maps(x, Wq, bq, Wk, bk, Wv, bv, Wo):
    tri_np = _tri_np()
    ones_np = np.ones((1, 128), dtype=np.float32)
    xT_b = [np.ascontiguousarray(x[b].T) for b in range(B)]
    in_maps = []
    for c in range(N_CORES):
        b, tp = divmod(c, TPG)
        sl = slice(tp * CH, (tp + 1) * CH)
        wv_aug = np.zeros((D, CHA), dtype=np.float32)
        bv_aug = np.zeros((1, CHA), dtype=np.float32)
        for h in range(HPC):
            hsl = slice(tp * CH + h * DH, tp * CH + (h + 1) * DH)
            wv_aug[:, h * 65:h * 65 + DH] = Wv[:, hsl]
            bv_aug[0, h * 65:h * 65 + DH] = bv[hsl]
            bv_aug[0, h * 65 + DH] = 1.0
        in_maps.append({
            "xT": xT_b[b],
            "wq": np.ascontiguousarray(Wq[:, sl], dtype=np.float32),
            "wk": np.ascontiguousarray(Wk[:, sl], dtype=np.float32),
            "wv": wv_aug,
            "wo": np.ascontiguousarray(Wo[sl, :], dtype=np.float32),
            "bq": (bq[sl].astype(np.float32) * 0.125).reshape(2, 128).T.copy(),
            "bk": bk[sl].astype(np.float32).reshape(2, 128).T.copy(),
            "bv": bv_aug,
            "tri": tri_np,
            "ones": ones_np,
        })
    return in_maps


def _get_program():
    global _PROG
    if _PROG is None:
        _PROG = _build_program()
    return _PROG


def kernel(x, mask, Wq, bq, Wk, bk, Wv, bv, Wo, bo):
    x = np.asarray(x, dtype=np.float32)
    mask = np.asarray(mask)
    Wq, Wk, Wv, Wo = (np.asarray(w, dtype=np.float32)
                      for w in (Wq, Wk, Wv, Wo))
    bq, bk, bv, bo = (np.asarray(b, dtype=np.float32)
                      for b in (bq, bk, bv, bo))
    causal = bool(
        np.array_equal(mask != 0,
                       np.tril(np.ones((S, S), dtype=bool))))
    if not causal:
        # Fallback for non-causal masks: exact host computation.
        q = (x @ Wq + bq).reshape(B, S, H, DH).transpose(0, 2, 1, 3)
        k = (x @ Wk + bk).reshape(B, S, H, DH).transpose(0, 2, 1, 3)
        v = (x @ Wv + bv).reshape(B, S, H, DH).transpose(0, 2, 1, 3)
        attn = np.einsum("bhqd,bhkd->bhqk", q, k) / np.sqrt(np.float32(DH))
        attn = np.where(mask == 0, np.float32(-1e9), attn)
        attn = attn - attn.max(axis=-1, keepdims=True)
        e = np.exp(attn)
        p = e / e.sum(axis=-1, keepdims=True)
        o = np.einsum("bhqk,bhkd->bhqd", p, v)
        o = o.transpose(0, 2, 1, 3).reshape(B, S, D)
        return (o @ Wo + bo).astype(np.float32)

    nc = _get_program()
    in_maps = build_in_maps(x, Wq, bq, Wk, bk, Wv, bv, Wo)
    res = run_bass_kernel_spmd(nc, in_maps, core_ids=list(range(N_CORES)))
    out = np.zeros((B, S, D), dtype=np.float32)
    for c in range(N_CORES):
        out[c // TPG] += res.results[c]["out"]
    out += bo.astype(np.float32)
    return out
